# revision 1
# baseline (speedup 1.0000x reference)
"""GRU-D Trainium2 Bass kernel.

Strategy: data-parallel over batch across 8 NeuronCores (B=256 -> 32/core).
Per core, layout is [H(partitions), B(free)] throughout.

Phase 1 (per 32-step chunk, overlapped with DMA): elementwise imputation
x_hat, decay dxt (ACT exp/relu), and delta_h = exp(-relu(W_gh@Delta+b_gh))
via matmul, all T stored in SBUF.

Phase 2 (per 8-step PSUM group): gate biases + input-dependent gate terms
A_z/A_r/A_h are matmul-accumulated into PSUM banks; the sequential scan then
accumulates U_*@g on top (start=False), reads gates out with one sigmoid ACT
([z|r] across 2 banks) + one tanh ACT, and updates h with DVE/Pool ops.

Matmuls run in bf16 (fp32 PSUM accumulate); h state stays fp32.
"""

import sys

sys.path.insert(0, "/opt/trn_rl_repo")

import contextlib
import ctypes
import types

import numpy as np

# ---------------------------------------------------------------- axon shim
_SO_PATH = "/opt/axon/libaxon_pjrt.so"


def _install_shims():
    if "antenv.axon_hooks" not in sys.modules:
        mod = types.ModuleType("antenv.axon_hooks")

        def _make_hook():
            try:
                lib = ctypes.CDLL(_SO_PATH)
            except OSError:
                return None
            if not hasattr(lib, "axon_start_nrt_profile"):
                return None
            lib.axon_start_nrt_profile.argtypes = [
                ctypes.POINTER(ctypes.c_int64),
                ctypes.c_size_t,
            ]
            lib.axon_start_nrt_profile.restype = ctypes.c_int64
            lib.axon_stop_nrt_profile.argtypes = [ctypes.c_char_p]
            lib.axon_stop_nrt_profile.restype = ctypes.c_int64

            @contextlib.contextmanager
            def _hook(output_dir, device_ids=None):
                import jax

                jax.devices()
                if device_ids:
                    ids = (ctypes.c_int64 * len(device_ids))(*device_ids)
                    rc = lib.axon_start_nrt_profile(ids, len(device_ids))
                else:
                    rc = lib.axon_start_nrt_profile(None, 0)
                if rc != 0:
                    raise RuntimeError(f"axon_start_nrt_profile rc={rc}")
                try:
                    yield
                finally:
                    n = lib.axon_stop_nrt_profile(str(output_dir).encode())
                    print(f"ntff profile: {n} file(s) -> {output_dir}", file=sys.stderr)

            return _hook

        hook = _make_hook()
        mod.get_axon_ntff_profile_hook = lambda: hook
        mod.set_axon_ntff_profile_hook = lambda h: None
        sys.modules["antenv.axon_hooks"] = mod

    import concourse.bass_utils as bu

    bu.upload_artifacts = lambda tmpdir: tmpdir


_install_shims()

import concourse.bass as bass
import concourse.bacc as bacc
import concourse.tile as tile
from concourse import mybir
from concourse.bass_utils import run_bass_kernel_spmd

F32 = mybir.dt.float32
BF16 = mybir.dt.bfloat16
AF = mybir.ActivationFunctionType

B, T, D, H = 256, 256, 128, 256
NCORES = 8
BC = B // NCORES  # 32 batch rows per core
COLS = T * BC  # 8192 sbuf columns (t-major, b minor)
TC1 = 32  # phase-1 chunk: 32 timesteps -> 1024 cols
NCH = T // TC1  # 8 chunks
TG = 8  # phase-2 group: 8 timesteps per PSUM bank set
NG = T // TG  # 32 groups
GCOLS = TG * BC  # 256

MAX_WAITS = 2

# ------------------------------------------------------- sync-wait limiting


def _cap_instruction_waits(nc):
    """Walrus rejects TPB instructions with too many sync waits.  Move excess
    waits onto earlier same-engine instructions.  Strictly we only move waits
    past instructions without sem updates; DMA-queue-sem waits (whose
    producers are triggered well before and cannot depend on this engine's
    nearby updates) may move past updaters."""
    import bisect

    f = nc.m.functions[0]
    for blk in f.blocks:
        insts = list(blk.instructions)
        # cumulative sem-update history in scheduled order
        semhist = {}  # sem -> ([pos...], [cumval...])
        cum = {}
        for pos, inst in enumerate(insts):
            si = inst.sync_info
            if si:
                for u in si.on_update:
                    v = cum.get(u.ant_name, 0) + (u.update_value or 1)
                    cum[u.ant_name] = v
                    h = semhist.setdefault(u.ant_name, ([], []))
                    h[0].append(pos)
                    h[1].append(v)

        def producer_pos(w):
            h = semhist.get(w.ant_name)
            if h is None:
                return -1  # produced outside this block (earlier) — movable
            i = bisect.bisect_left(h[1], w.wait_value)
            if i >= len(h[1]):
                return 1 << 60
            return h[0][i]

        prev_by_engine = {}
        seen_ge = {}  # (engine, sem) -> max threshold already waited on
        for pos, inst in enumerate(insts):
            si = inst.sync_info
            waits = list(si.on_wait) if si else []
            if len(waits) > MAX_WAITS:
                # ACT and DVE execute strictly in order (DVE even drains its
                # pipe between ops), so a wait on the engine's own compute
                # semaphore is enforced by program order already — drop it.
                ename = str(inst.engine).split(".")[-1]
                if ename in ("Activation", "DVE"):
                    kept = [
                        w
                        for w in waits
                        if not (
                            str(w.wait_mode) == "sem-ge-imm"
                            and w.ant_name.startswith(ename + "_")
                        )
                    ]
                    if len(kept) < len(waits):
                        waits = kept
                        si.on_wait = waits
                        inst.sync_info = si
            if len(waits) > MAX_WAITS:
                # drop waits dominated by an earlier same-engine wait
                kept = []
                for w in waits:
                    if (
                        str(w.wait_mode) == "sem-ge-imm"
                        and seen_ge.get((inst.engine, w.ant_name), -1) >= w.wait_value
                    ):
                        continue
                    kept.append(w)
                if len(kept) < len(waits):
                    waits = kept
                    si.on_wait = waits
                    inst.sync_info = si
            if len(waits) > MAX_WAITS:
                # merge same-sem ge-waits, keeping the max threshold
                merged, ok = {}, True
                for w in waits:
                    key = w.ant_name
                    if str(w.wait_mode) != "sem-ge-imm":
                        key, ok = (w.ant_name, len(merged)), False
                    if key not in merged or w.wait_value > merged[key].wait_value:
                        merged[key] = w
                if ok and len(merged) < len(waits):
                    waits = list(merged.values())
                    si.on_wait = waits
                    inst.sync_info = si
            if len(waits) > MAX_WAITS and type(inst).__name__ != "InstDMACopy":
                keep, excess = waits[:MAX_WAITS], waits[MAX_WAITS:]
                si.on_wait = keep
                inst.sync_info = si
                for jpos, p in reversed(prev_by_engine.get(inst.engine, [])):
                    if not excess:
                        break
                    movable = [w for w in excess if producer_pos(w) < jpos]
                    if not movable:
                        continue
                    psi = p.sync_info
                    pw = list(psi.on_wait) if psi else []
                    room = MAX_WAITS - len(pw)
                    if room > 0:
                        take = movable[:room]
                        if psi is None:
                            psi = mybir.SyncInfo(on_wait=[], on_update=[])
                        psi.on_wait = pw + take
                        p.sync_info = psi
                        tk = {(w.ant_name, w.wait_value) for w in take}
                        excess = [
                            w for w in excess if (w.ant_name, w.wait_value) not in tk
                        ]
                if excess:
                    raise RuntimeError(
                        f"could not place {len(excess)} waits for {inst.name} "
                        f"({type(inst).__name__}) "
                        f"{[(w.ant_name, w.wait_value) for w in excess]}"
                    )
            final_si = inst.sync_info
            if final_si:
                for w in final_si.on_wait:
                    if str(w.wait_mode) == "sem-ge-imm":
                        key = (inst.engine, w.ant_name)
                        if w.wait_value > seen_ge.get(key, -1):
                            seen_ge[key] = w.wait_value
            prev_by_engine.setdefault(inst.engine, []).append((pos, inst))


def _patch_drain_and_barrier():
    """The kernel-tail drain waits on every live semaphore; spread the waits
    over trailing nops so each instruction stays within the ISA limit."""
    if getattr(tile.TileContext, "_drain_patched", False):
        return
    ScopedClock = tile.ScopedClock

    def _drain_and_barrier(self, tick_clock, wait_clock):
        drain_inst = self.nc.sync.drain()
        wait_clock.add_sem_waits(
            drain_inst.ins, ScopedClock({None: tick_clock.global_clock})
        )
        si = drain_inst.ins.sync_info
        waits = list(si.on_wait) if si else []
        if len(waits) > MAX_WAITS:
            si.on_wait = waits[:MAX_WAITS]
            drain_inst.ins.sync_info = si
            rest = waits[MAX_WAITS:]
            while rest:
                chunk, rest = rest[:MAX_WAITS], rest[MAX_WAITS:]
                nop = self.nc.sync.nop(nofuse=True)
                nsi = nop.ins.sync_info
                if nsi is None:
                    nsi = mybir.SyncInfo(on_wait=[], on_update=[])
                nsi.on_wait = chunk
                nop.ins.sync_info = nsi

        self.nc.all_engine_barrier()
        assert self.sems is not None
        popped = self.nc._tile_sem_poison_stack.pop()
        assert popped is self._sem_poison
        self.nc.clear_and_free_semaphores(list(self.sems.allocated().values()))
        self.nc.all_engine_barrier()

    tile.TileContext._drain_and_barrier = _drain_and_barrier
    tile.TileContext._drain_patched = True


# ------------------------------------------------------------ build program

_BUILT = None


def _build():
    global _BUILT
    if _BUILT is not None:
        return _BUILT

    nc = bacc.Bacc("TRN2", target_bir_lowering=False, debug=False)

    inp4 = nc.dram_tensor("inp4", [4, D, COLS], F32, kind="ExternalInput")
    xmean_t = nc.dram_tensor("xmean_t", [D, T], F32, kind="ExternalInput")
    wgx_diag = nc.dram_tensor("wgx_diag", [D, 1], F32, kind="ExternalInput")
    b_gx_c = nc.dram_tensor("b_gx_c", [D, 1], F32, kind="ExternalInput")
    wgh_t = nc.dram_tensor("wgh_t", [D, H], BF16, kind="ExternalInput")
    b_gh2 = nc.dram_tensor("b_gh2", [128, 2], F32, kind="ExternalInput")
    gates = {}
    for gname in ("z", "r", "h"):
        gates[gname] = dict(
            wx=nc.dram_tensor(f"wx_{gname}", [D, H], BF16, kind="ExternalInput"),
            wm=nc.dram_tensor(f"wm_{gname}", [D, H], BF16, kind="ExternalInput"),
            u=nc.dram_tensor(f"u_{gname}", [128, 2, 2, 128], BF16, kind="ExternalInput"),
            b2=nc.dram_tensor(f"b2_{gname}", [2, 128], BF16, kind="ExternalInput"),
        )
    ones2 = nc.dram_tensor("ones2", [2, 512], BF16, kind="ExternalInput")
    wout2 = nc.dram_tensor("wout2", [128, 2], F32, kind="ExternalInput")
    b_out_c = nc.dram_tensor("b_out_c", [1, 1], F32, kind="ExternalInput")
    out_d = nc.dram_tensor("out", [1, BC], F32, kind="ExternalOutput")

    with tile.TileContext(nc) as tc:
        with contextlib.ExitStack() as ctx:
            const = ctx.enter_context(tc.tile_pool(name="const", bufs=1))
            persist = ctx.enter_context(tc.tile_pool(name="persist", bufs=1))
            ph1 = ctx.enter_context(tc.tile_pool(name="ph1", bufs=2))
            tmp1 = ctx.enter_context(tc.tile_pool(name="tmp1", bufs=3))
            scan = ctx.enter_context(tc.tile_pool(name="scan", bufs=6))
            ps_zr = ctx.enter_context(tc.tile_pool(name="ps_zr", bufs=2, space="PSUM"))
            ps_h = ctx.enter_context(tc.tile_pool(name="ps_h", bufs=2, space="PSUM"))
            ps_dht = ctx.enter_context(tc.tile_pool(name="ps_dht", bufs=1, space="PSUM"))
            ps_out = ctx.enter_context(tc.tile_pool(name="ps_out", bufs=1, space="PSUM"))

            # landing pads for relocated sem waits (see _cap_instruction_waits)
            for eng in (nc.scalar, nc.vector, nc.gpsimd, nc.tensor):
                for _ in range(4):
                    eng.nop(nofuse=True)

            # ---- constants to SBUF
            def cload(drt, shape, dt):
                t = const.tile(shape, dt, tag=drt.name)
                nc.gpsimd.dma_start(out=t, in_=drt[...])
                return t

            s_xmean = cload(xmean_t, [D, T], F32)
            s_wgxd = cload(wgx_diag, [D, 1], F32)
            s_bgx = cload(b_gx_c, [D, 1], F32)
            s_wgh = cload(wgh_t, [D, H], BF16)
            s_bgh2 = cload(b_gh2, [128, 2], F32)
            s_g = {}
            for gname, gd in gates.items():
                s_g[gname] = dict(
                    wx=cload(gd["wx"], [D, H], BF16),
                    wm=cload(gd["wm"], [D, H], BF16),
                    u=cload(gd["u"], [128, 2, 2, 128], BF16),
                    b2=cload(gd["b2"], [2, 128], BF16),
                )
            s_ones2 = cload(ones2, [2, 512], BF16)
            s_wout2 = cload(wout2, [128, 2], F32)
            s_bout = cload(b_out_c, [1, 1], F32)

            xhat_bf = persist.tile([D, COLS], BF16)
            m_bf = persist.tile([D, COLS], BF16)
            dht = persist.tile([128, 2, COLS], F32)

            # =========================== phase 1 ===========================
            CH = TC1 * BC  # 1024
            for c in range(NCH):
                cs = c * CH
                x_t = ph1.tile([D, CH], F32, tag="x")
                xl_t = ph1.tile([D, CH], F32, tag="xl")
                mk_t = ph1.tile([D, CH], F32, tag="mk")
                dl_t = ph1.tile([D, CH], F32, tag="dl")
                for ch, tt in ((0, x_t), (1, xl_t), (2, mk_t), (3, dl_t)):
                    nc.sync.dma_start(out=tt, in_=inp4[ch, :, cs : cs + CH])

                # 1-element anchored reads: give ACT/DVE an early wait on the
                # chunk's DMA sems so real ops' waits collapse by dominance
                padA = tmp1.tile([1, 1], F32, tag="padA")
                nc.scalar.copy(padA, dl_t[:1, :1])
                padV = tmp1.tile([1, 1], F32, tag="padV")
                nc.vector.tensor_copy(padV, xl_t[:1, :1])
                padV2 = tmp1.tile([1, 1], F32, tag="padV2")
                nc.vector.tensor_copy(padV2, x_t[:1, :1])
                padV3 = tmp1.tile([1, 1], F32, tag="padV3")
                nc.vector.tensor_copy(padV3, mk_t[:1, :1])
                padP = tmp1.tile([1, 1], F32, tag="padP")
                nc.gpsimd.tensor_copy(padP, mk_t[:1, :1])
                padP2 = tmp1.tile([1, 1], F32, tag="padP2")
                nc.gpsimd.tensor_copy(padP2, dl_t[:1, :1])

                # xm broadcast AP: [D, TC1(t), BC(b)] with b-step 0
                xsl = s_xmean[:, c * TC1 : (c + 1) * TC1]
                xm_b = bass.AP(
                    tensor=xsl.tensor,
                    offset=xsl.offset,
                    ap=[xsl.ap[0], xsl.ap[1], [0, BC]],
                )

                def r3(t):
                    return t.rearrange("p (t b) -> p t b", b=BC)

                # dxt = exp(-relu(wgx*Delta + bgx))
                u_t = tmp1.tile([D, CH], F32, tag="t1")
                nc.scalar.activation(u_t, dl_t, AF.Relu, bias=s_bgx[:, 0:1], scale=s_wgxd[:, 0:1])
                dxt = tmp1.tile([D, CH], F32, tag="t2")
                nc.scalar.activation(dxt, u_t, AF.Exp, scale=-1.0)

                # imputation: s3 = xm + dxt*(xl-xm); xhat = s3 + m*(x-s3)
                s1 = tmp1.tile([D, CH], F32, tag="t1")
                nc.vector.tensor_sub(r3(s1), r3(xl_t), xm_b)
                s2 = tmp1.tile([D, CH], F32, tag="t3")
                nc.vector.tensor_mul(s2, dxt, s1)
                s3 = tmp1.tile([D, CH], F32, tag="t1")
                nc.vector.tensor_add(r3(s3), r3(s2), xm_b)
                s4 = tmp1.tile([D, CH], F32, tag="t2")
                nc.vector.tensor_sub(s4, x_t, s3)
                s5 = tmp1.tile([D, CH], F32, tag="t3")
                nc.vector.tensor_mul(s5, mk_t, s4)
                nc.vector.tensor_add(xhat_bf[:, cs : cs + CH], s3, s5)

                # bf16 copies for matmul rhs
                nc.gpsimd.tensor_copy(m_bf[:, cs : cs + CH], mk_t)
                dl_bf = tmp1.tile([D, CH], BF16, tag="t4")
                nc.gpsimd.tensor_copy(dl_bf, dl_t)

                # delta_h = exp(-relu(W_gh @ Delta + b_gh))
                for mi in range(2):
                    for ni in range(2):
                        pd = ps_dht.tile([128, 512], F32)
                        nc.tensor.matmul(
                            pd,
                            s_wgh[:, mi * 128 : (mi + 1) * 128],
                            dl_bf[:, ni * 512 : (ni + 1) * 512],
                            start=True,
                            stop=True,
                        )
                        rl = tmp1.tile([128, 512], F32, tag="t5")
                        nc.scalar.activation(rl, pd, AF.Relu, bias=s_bgh2[:, mi : mi + 1])
                        nc.scalar.activation(
                            dht[:, mi, cs + ni * 512 : cs + (ni + 1) * 512],
                            rl,
                            AF.Exp,
                            scale=-1.0,
                        )

            # =========================== phase 2 ===========================
            h_st = persist.tile([128, 2, BC], F32)
            nc.vector.memset(h_st, 0.0)

            for g in range(NG):
                gs = g * GCOLS
                pzr = ps_zr.tile([128, 1024], F32)  # banks: z | r
                ph_ = ps_h.tile([128, 512], F32)

                # biases (start=True clears banks)
                nc.tensor.matmul(pzr[:, 0:512], s_g["z"]["b2"], s_ones2, start=True, stop=False, skip_group_check=True)
                nc.tensor.matmul(pzr[:, 512:1024], s_g["r"]["b2"], s_ones2, start=True, stop=False, skip_group_check=True)
                nc.tensor.matmul(ph_[:, 0:512], s_g["h"]["b2"], s_ones2, start=True, stop=False, skip_group_check=True)

                # input-dependent gate terms, N=256 per (gate, m-tile)
                for gname, dst, goff in (("z", pzr, 0), ("r", pzr, 512), ("h", ph_, 0)):
                    sg = s_g[gname]
                    for mi in range(2):
                        reg = dst[:, goff + mi * 256 : goff + (mi + 1) * 256]
                        nc.tensor.matmul(
                            reg, sg["wx"][:, mi * 128 : (mi + 1) * 128],
                            xhat_bf[:, gs : gs + GCOLS],
                            start=False, stop=False, skip_group_check=True,
                        )
                        nc.tensor.matmul(
                            reg, sg["wm"][:, mi * 128 : (mi + 1) * 128],
                            m_bf[:, gs : gs + GCOLS],
                            start=False, stop=False, skip_group_check=True,
                        )

                pzr4 = pzr.rearrange("p (j q b) -> p j q b", j=4, b=BC)
                ph2 = ph_.rearrange("p (j q b) -> p j q b", j=2, b=BC)

                for tl in range(TG):
                    t = g * TG + tl
                    # g_t = dht_t * h
                    gcur = scan.tile([128, 2, BC], F32, tag="g")
                    nc.vector.tensor_mul(gcur, h_st, dht[:, :, t * BC : (t + 1) * BC])
                    gbf = scan.tile([128, 2, BC], BF16, tag="gbf")
                    nc.vector.tensor_copy(gbf, gcur)

                    # z/r recurrent matmuls accumulate onto gate banks
                    for gname, goff in (("z", 0), ("r", 2)):
                        uu = s_g[gname]["u"]
                        for mi in range(2):
                            reg = pzr4[:, goff + mi, tl, :]
                            for k in range(2):
                                nc.tensor.matmul(
                                    reg, uu[:, k, mi, :], gbf[:, k, :],
                                    start=False, stop=(k == 1), skip_group_check=True,
                                )
                    zr = scan.tile([128, 4, BC], F32, tag="zr")
                    nc.scalar.activation(zr, pzr4[:, :, tl, :], AF.Sigmoid)

                    sbf = scan.tile([128, 2, BC], BF16, tag="sbf")
                    nc.vector.tensor_mul(sbf, zr[:, 2:4, :], gcur)

                    uu = s_g["h"]["u"]
                    for mi in range(2):
                        reg = ph2[:, mi, tl, :]
                        for k in range(2):
                            nc.tensor.matmul(
                                reg, uu[:, k, mi, :], sbf[:, k, :],
                                start=False, stop=(k == 1), skip_group_check=True,
                            )
                    c_t = scan.tile([128, 2, BC], F32, tag="c")
                    nc.scalar.activation(c_t, ph2[:, :, tl, :], AF.Tanh)

                    d_t = scan.tile([128, 2, BC], F32, tag="d")
                    nc.vector.tensor_sub(d_t, c_t, gcur)
                    p_t = scan.tile([128, 2, BC], F32, tag="p")
                    nc.vector.tensor_mul(p_t, zr[:, 0:2, :], d_t)
                    nc.vector.tensor_add(h_st, gcur, p_t)

            # ---- output: out = W_out @ h + b_out  -> [1, BC]
            po = ps_out.tile([1, BC], F32)
            for k in range(2):
                nc.tensor.matmul(
                    po, s_wout2[:, k : k + 1], h_st[:, k, :],
                    start=(k == 0), stop=(k == 1), skip_group_check=True,
                )
            o_sb = scan.tile([1, BC], F32, tag="o")
            nc.scalar.activation(o_sb, po, AF.Identity, bias=s_bout[:, 0:1])
            nc.sync.dma_start(out=out_d[:, :], in_=o_sb)

    nc.compile()  # bacc: splits multi-sem waits into event-semaphore chains
    _BUILT = nc
    return nc


# ------------------------------------------------------------- host wrapper

TRACE = False
LAST_EXEC_NS = None
LAST_RESULT = None


def _host_prep(inputs):
    import ml_dtypes

    bf = ml_dtypes.bfloat16
    inp = np.asarray(inputs["inp"], np.float32)
    X_mean = np.asarray(inputs["X_mean"], np.float32)
    W_z = np.asarray(inputs["W_z"], np.float32)
    b_z = np.asarray(inputs["b_z"], np.float32)
    W_r = np.asarray(inputs["W_r"], np.float32)
    b_r = np.asarray(inputs["b_r"], np.float32)
    W_h = np.asarray(inputs["W_h"], np.float32)
    b_h = np.asarray(inputs["b_h"], np.float32)
    W_gx = np.asarray(inputs["W_gx"], np.float32)
    b_gx = np.asarray(inputs["b_gx"], np.float32)
    W_gh = np.asarray(inputs["W_gh"], np.float32)
    b_gh = np.asarray(inputs["b_gh"], np.float32)
    W_out = np.asarray(inputs["W_out"], np.float32)
    b_out = np.asarray(inputs["b_out"], np.float32)

    def uprep(W):
        U = W[:, D : D + H]  # [256, 256]
        return np.ascontiguousarray(
            U.reshape(2, 128, 2, 128).transpose(3, 2, 0, 1)
        ).astype(bf)

    shared = {
        "xmean_t": np.ascontiguousarray(X_mean[0].T),
        "wgx_diag": np.ascontiguousarray(np.diag(W_gx)).reshape(D, 1),
        "b_gx_c": b_gx.reshape(D, 1),
        "wgh_t": np.ascontiguousarray(W_gh.T).astype(bf),
        "b_gh2": np.ascontiguousarray(b_gh.reshape(2, 128).T),
        "ones2": np.concatenate(
            [
                np.concatenate([np.ones((1, 256)), np.zeros((1, 256))], 1),
                np.concatenate([np.zeros((1, 256)), np.ones((1, 256))], 1),
            ],
            0,
        ).astype(bf),
        "wout2": np.ascontiguousarray(W_out[0].reshape(2, 128).T),
        "b_out_c": b_out.reshape(1, 1),
    }
    for gname, W, bv in (("z", W_z, b_z), ("r", W_r, b_r), ("h", W_h, b_h)):
        shared[f"wx_{gname}"] = np.ascontiguousarray(W[:, :D].T).astype(bf)
        shared[f"wm_{gname}"] = np.ascontiguousarray(W[:, D + H :].T).astype(bf)
        shared[f"u_{gname}"] = uprep(W)
        shared[f"b2_{gname}"] = bv.reshape(2, 128).astype(bf)

    in_maps = []
    for c in range(NCORES):
        sl = inp[c * BC : (c + 1) * BC]  # [BC, 4, T, D]
        arr = np.ascontiguousarray(sl.transpose(1, 3, 2, 0)).reshape(4, D, COLS)
        m = dict(shared)
        m["inp4"] = arr
        in_maps.append(m)
    return in_maps


def kernel(**inputs):
    global LAST_EXEC_NS, LAST_RESULT
    nc = _build()
    in_maps = _host_prep(inputs)
    res = run_bass_kernel_spmd(nc, in_maps, list(range(NCORES)), trace=TRACE)
    LAST_EXEC_NS = res.exec_time_ns
    LAST_RESULT = res
    out = np.concatenate([res.results[c]["out"][0] for c in range(NCORES)])
    return out.reshape(B, 1).astype(np.float32)



# revision 7
# speedup vs baseline: 1.2996x; 1.2996x over previous
"""GRU-D Trainium2 Bass kernel.

Strategy: data-parallel over batch across 8 NeuronCores (B=256 -> 32/core).
Per core, layout is [H(partitions), B(free)] throughout.

The scan is latency-bound (serial dependency chain per timestep), so the
kernel is organized around shortening the per-step critical path:

  gbf -> PE r-matmuls -> sigmoid(r) -> sbf=r*g -> PE h-matmuls -> tanh
      -> u = f*c -> gbf' = u - e_neg            (2 DVE ops after tanh)

using the identity  g' = dht'*h = (dht'*z)*c + (1-z)*(dht'*g)
                       = f*c - (z-1)*w  with  w = dht'*g.
f, e_neg=(z-1)*w, w are computed OFF the chain (DVE scalar_tensor_tensor /
Pool engine) while PE/ACT work.  The fp32 state g' is produced on Pool.

All activation functions used (sigmoid/tanh/relu/copy/identity) live in ONE
ACT table set; exp() is eliminated via exp(-relu(q)) = min(1/sigmoid(q)-1, 1)
with DVE reciprocal_approx_fast, so there are no table reloads.

PSUM group preambles (gate bias + input-dependent terms, 15 matmuls) for
group g+1 and phase-1 chunk work (imputation x_hat, decay dht) are
interleaved into the idle engine slots of the running scan.

Matmuls run in bf16 (fp32 PSUM accumulate); h/g state stays fp32.
"""

import sys

sys.path.insert(0, "/opt/trn_rl_repo")

import contextlib
import ctypes
import types

import numpy as np

# ---------------------------------------------------------------- axon shim
_SO_PATH = "/opt/axon/libaxon_pjrt.so"


def _install_shims():
    if "antenv.axon_hooks" not in sys.modules:
        mod = types.ModuleType("antenv.axon_hooks")

        def _make_hook():
            try:
                lib = ctypes.CDLL(_SO_PATH)
            except OSError:
                return None
            if not hasattr(lib, "axon_start_nrt_profile"):
                return None
            lib.axon_start_nrt_profile.argtypes = [
                ctypes.POINTER(ctypes.c_int64),
                ctypes.c_size_t,
            ]
            lib.axon_start_nrt_profile.restype = ctypes.c_int64
            lib.axon_stop_nrt_profile.argtypes = [ctypes.c_char_p]
            lib.axon_stop_nrt_profile.restype = ctypes.c_int64

            @contextlib.contextmanager
            def _hook(output_dir, device_ids=None):
                import jax

                jax.devices()
                if device_ids:
                    ids = (ctypes.c_int64 * len(device_ids))(*device_ids)
                    rc = lib.axon_start_nrt_profile(ids, len(device_ids))
                else:
                    rc = lib.axon_start_nrt_profile(None, 0)
                if rc != 0:
                    raise RuntimeError(f"axon_start_nrt_profile rc={rc}")
                try:
                    yield
                finally:
                    n = lib.axon_stop_nrt_profile(str(output_dir).encode())
                    print(f"ntff profile: {n} file(s) -> {output_dir}", file=sys.stderr)

            return _hook

        hook = _make_hook()
        mod.get_axon_ntff_profile_hook = lambda: hook
        mod.set_axon_ntff_profile_hook = lambda h: None
        sys.modules["antenv.axon_hooks"] = mod

    import concourse.bass_utils as bu

    bu.upload_artifacts = lambda tmpdir: tmpdir


_install_shims()

import concourse.bass as bass
import concourse.bacc as bacc
import concourse.tile as tile
from concourse import mybir
from concourse.bass_utils import run_bass_kernel_spmd

F32 = mybir.dt.float32
BF16 = mybir.dt.bfloat16
AF = mybir.ActivationFunctionType
ALU = mybir.AluOpType

B, T, D, H = 256, 256, 128, 256
NCORES = 8
BC = B // NCORES  # 32 batch rows per core
COLS = T * BC  # 8192 sbuf columns (t-major, b minor)
COLSP = COLS + BC  # +1 virtual step of dht==1 so g'_(T-1) == h_(T-1)
TC1 = 16  # phase-1 chunk: 16 timesteps -> 512 cols
NCH = T // TC1  # 16 chunks
CH = TC1 * BC  # 512
TG = 8  # phase-2 group: 8 timesteps per PSUM bank set
NG = T // TG  # 32 groups
GCOLS = TG * BC  # 256

MAX_WAITS = 2

# ------------------------------------------------------- sync-wait limiting


def _cap_instruction_waits(nc):
    """Walrus rejects TPB instructions with too many sync waits.  Move excess
    waits onto earlier same-engine instructions.  Strictly we only move waits
    past instructions without sem updates; DMA-queue-sem waits (whose
    producers are triggered well before and cannot depend on this engine's
    nearby updates) may move past updaters."""
    import bisect

    f = nc.m.functions[0]
    for blk in f.blocks:
        insts = list(blk.instructions)
        # cumulative sem-update history in scheduled order
        semhist = {}  # sem -> ([pos...], [cumval...])
        cum = {}
        for pos, inst in enumerate(insts):
            si = inst.sync_info
            if si:
                for u in si.on_update:
                    v = cum.get(u.ant_name, 0) + (u.update_value or 1)
                    cum[u.ant_name] = v
                    h = semhist.setdefault(u.ant_name, ([], []))
                    h[0].append(pos)
                    h[1].append(v)

        def producer_pos(w):
            h = semhist.get(w.ant_name)
            if h is None:
                return -1  # produced outside this block (earlier) — movable
            i = bisect.bisect_left(h[1], w.wait_value)
            if i >= len(h[1]):
                return 1 << 60
            return h[0][i]

        prev_by_engine = {}
        seen_ge = {}  # (engine, sem) -> max threshold already waited on
        for pos, inst in enumerate(insts):
            si = inst.sync_info
            waits = list(si.on_wait) if si else []
            if len(waits) > MAX_WAITS:
                # ACT and DVE execute strictly in order (DVE even drains its
                # pipe between ops), so a wait on the engine's own compute
                # semaphore is enforced by program order already — drop it.
                ename = str(inst.engine).split(".")[-1]
                if ename in ("Activation", "DVE"):
                    kept = [
                        w
                        for w in waits
                        if not (
                            str(w.wait_mode) == "sem-ge-imm"
                            and w.ant_name.startswith(ename + "_")
                        )
                    ]
                    if len(kept) < len(waits):
                        waits = kept
                        si.on_wait = waits
                        inst.sync_info = si
            if len(waits) > MAX_WAITS:
                # drop waits dominated by an earlier same-engine wait
                kept = []
                for w in waits:
                    if (
                        str(w.wait_mode) == "sem-ge-imm"
                        and seen_ge.get((inst.engine, w.ant_name), -1) >= w.wait_value
                    ):
                        continue
                    kept.append(w)
                if len(kept) < len(waits):
                    waits = kept
                    si.on_wait = waits
                    inst.sync_info = si
            if len(waits) > MAX_WAITS:
                # merge same-sem ge-waits, keeping the max threshold
                merged, ok = {}, True
                for w in waits:
                    key = w.ant_name
                    if str(w.wait_mode) != "sem-ge-imm":
                        key, ok = (w.ant_name, len(merged)), False
                    if key not in merged or w.wait_value > merged[key].wait_value:
                        merged[key] = w
                if ok and len(merged) < len(waits):
                    waits = list(merged.values())
                    si.on_wait = waits
                    inst.sync_info = si
            if len(waits) > MAX_WAITS and type(inst).__name__ != "InstDMACopy":
                keep, excess = waits[:MAX_WAITS], waits[MAX_WAITS:]
                si.on_wait = keep
                inst.sync_info = si
                for jpos, p in reversed(prev_by_engine.get(inst.engine, [])):
                    if not excess:
                        break
                    movable = [w for w in excess if producer_pos(w) < jpos]
                    if not movable:
                        continue
                    psi = p.sync_info
                    pw = list(psi.on_wait) if psi else []
                    room = MAX_WAITS - len(pw)
                    if room > 0:
                        take = movable[:room]
                        if psi is None:
                            psi = mybir.SyncInfo(on_wait=[], on_update=[])
                        psi.on_wait = pw + take
                        p.sync_info = psi
                        tk = {(w.ant_name, w.wait_value) for w in take}
                        excess = [
                            w for w in excess if (w.ant_name, w.wait_value) not in tk
                        ]
                if excess:
                    raise RuntimeError(
                        f"could not place {len(excess)} waits for {inst.name} "
                        f"({type(inst).__name__}) "
                        f"{[(w.ant_name, w.wait_value) for w in excess]}"
                    )
            final_si = inst.sync_info
            if final_si:
                for w in final_si.on_wait:
                    if str(w.wait_mode) == "sem-ge-imm":
                        key = (inst.engine, w.ant_name)
                        if w.wait_value > seen_ge.get(key, -1):
                            seen_ge[key] = w.wait_value
            prev_by_engine.setdefault(inst.engine, []).append((pos, inst))


def _patch_drain_and_barrier():
    """The kernel-tail drain waits on every live semaphore; spread the waits
    over trailing nops so each instruction stays within the ISA limit."""
    if getattr(tile.TileContext, "_drain_patched", False):
        return
    ScopedClock = tile.ScopedClock

    def _drain_and_barrier(self, tick_clock, wait_clock):
        drain_inst = self.nc.sync.drain()
        wait_clock.add_sem_waits(
            drain_inst.ins, ScopedClock({None: tick_clock.global_clock})
        )
        si = drain_inst.ins.sync_info
        waits = list(si.on_wait) if si else []
        if len(waits) > MAX_WAITS:
            si.on_wait = waits[:MAX_WAITS]
            drain_inst.ins.sync_info = si
            rest = waits[MAX_WAITS:]
            while rest:
                chunk, rest = rest[:MAX_WAITS], rest[MAX_WAITS:]
                nop = self.nc.sync.nop(nofuse=True)
                nsi = nop.ins.sync_info
                if nsi is None:
                    nsi = mybir.SyncInfo(on_wait=[], on_update=[])
                nsi.on_wait = chunk
                nop.ins.sync_info = nsi

        self.nc.all_engine_barrier()
        assert self.sems is not None
        popped = self.nc._tile_sem_poison_stack.pop()
        assert popped is self._sem_poison
        self.nc.clear_and_free_semaphores(list(self.sems.allocated().values()))
        self.nc.all_engine_barrier()

    tile.TileContext._drain_and_barrier = _drain_and_barrier
    tile.TileContext._drain_patched = True


# ------------------------------------------------------------ build program

_BUILT = None


def _build():
    global _BUILT
    if _BUILT is not None:
        return _BUILT

    nc = bacc.Bacc("TRN2", target_bir_lowering=False, debug=False)

    inp4 = nc.dram_tensor("inp4", [4, D, COLS], F32, kind="ExternalInput")
    xmean_t = nc.dram_tensor("xmean_t", [D, T], F32, kind="ExternalInput")
    wgx_diag = nc.dram_tensor("wgx_diag", [D, 1], F32, kind="ExternalInput")
    b_gx_c = nc.dram_tensor("b_gx_c", [D, 1], F32, kind="ExternalInput")
    wgh_t = nc.dram_tensor("wgh_t", [D, H], BF16, kind="ExternalInput")
    b_gh2 = nc.dram_tensor("b_gh2", [128, 2], F32, kind="ExternalInput")
    gates = {}
    for gname in ("z", "r", "h"):
        gates[gname] = dict(
            wx=nc.dram_tensor(f"wx_{gname}", [D, H], BF16, kind="ExternalInput"),
            wm=nc.dram_tensor(f"wm_{gname}", [D, H], BF16, kind="ExternalInput"),
            u=nc.dram_tensor(f"u_{gname}", [128, 2, 2, 128], BF16, kind="ExternalInput"),
            b2=nc.dram_tensor(f"b2_{gname}", [2, 128], BF16, kind="ExternalInput"),
        )
    ones2 = nc.dram_tensor("ones2", [2, 512], BF16, kind="ExternalInput")
    wout2 = nc.dram_tensor("wout2", [128, 2], F32, kind="ExternalInput")
    b_out_c = nc.dram_tensor("b_out_c", [1, 1], F32, kind="ExternalInput")
    out_d = nc.dram_tensor("out", [1, BC], F32, kind="ExternalOutput")

    with tile.TileContext(nc) as tc:
        with contextlib.ExitStack() as ctx:
            const = ctx.enter_context(tc.tile_pool(name="const", bufs=1))
            persist = ctx.enter_context(tc.tile_pool(name="persist", bufs=1))
            ph1 = ctx.enter_context(tc.tile_pool(name="ph1", bufs=3))
            tmp1 = ctx.enter_context(tc.tile_pool(name="tmp1", bufs=2))
            scan = ctx.enter_context(tc.tile_pool(name="scan", bufs=6))
            ps_zr = ctx.enter_context(tc.tile_pool(name="ps_zr", bufs=2, space="PSUM"))
            ps_h = ctx.enter_context(tc.tile_pool(name="ps_h", bufs=2, space="PSUM"))
            ps_dht = ctx.enter_context(tc.tile_pool(name="ps_dht", bufs=1, space="PSUM"))
            ps_out = ctx.enter_context(tc.tile_pool(name="ps_out", bufs=1, space="PSUM"))

            # landing pads for relocated sem waits (see _cap_instruction_waits)
            for eng in (nc.scalar, nc.vector, nc.gpsimd, nc.tensor):
                for _ in range(4):
                    eng.nop(nofuse=True)

            # ---- constants to SBUF
            def cload(drt, shape, dt):
                t = const.tile(shape, dt, tag=drt.name)
                nc.gpsimd.dma_start(out=t, in_=drt[...])
                return t

            s_xmean = cload(xmean_t, [D, T], F32)
            s_wgxd = cload(wgx_diag, [D, 1], F32)
            s_bgx = cload(b_gx_c, [D, 1], F32)
            s_wgh = cload(wgh_t, [D, H], BF16)
            s_bgh2 = cload(b_gh2, [128, 2], F32)
            s_g = {}
            for gname, gd in gates.items():
                s_g[gname] = dict(
                    wx=cload(gd["wx"], [D, H], BF16),
                    wm=cload(gd["wm"], [D, H], BF16),
                    u=cload(gd["u"], [128, 2, 2, 128], BF16),
                    b2=cload(gd["b2"], [2, 128], BF16),
                )
            s_ones2 = cload(ones2, [2, 512], BF16)
            s_wout2 = cload(wout2, [128, 2], F32)
            s_bout = cload(b_out_c, [1, 1], F32)

            xhat_bf = persist.tile([D, COLS], BF16)
            m_bf = persist.tile([D, COLS], BF16)
            dht = persist.tile([128, 2, COLSP], F32)
            # virtual step T: dht == 1 so the final g' equals h_(T-1)
            nc.vector.memset(dht[:, :, COLS:COLSP], 1.0)

            gbf0 = persist.tile([128, 2, BC], BF16)
            nc.vector.memset(gbf0, 0.0)
            g0 = persist.tile([128, 2, BC], F32)
            nc.vector.memset(g0, 0.0)

            # ======================= phase 1 pieces ========================
            ph1_tiles = {}

            def ph1_dma(c):
                cs = c * CH
                x_t = ph1.tile([D, CH], F32, tag="x")
                xl_t = ph1.tile([D, CH], F32, tag="xl")
                mk_t = ph1.tile([D, CH], F32, tag="mk")
                dl_t = ph1.tile([D, CH], F32, tag="dl")
                ph1_tiles[c] = (x_t, xl_t, mk_t, dl_t)
                for chn, tt in ((0, x_t), (1, xl_t), (2, mk_t), (3, dl_t)):
                    nc.sync.dma_start(out=tt, in_=inp4[chn, :, cs : cs + CH])

            def ph1_partA(c):
                """dht for chunk c: 8 emission slots of instruction lists.
                dht = exp(-relu(W_gh@Delta+b_gh)) = min(1/sigmoid(q)-1, 1)."""
                cs = c * CH
                x_t, xl_t, mk_t, dl_t = ph1_tiles[c]

                def pads():
                    padA = tmp1.tile([1, 1], F32, tag="padA")
                    nc.scalar.copy(padA, dl_t[:1, :1])
                    padV = tmp1.tile([1, 1], F32, tag="padV")
                    nc.vector.tensor_copy(padV, xl_t[:1, :1])
                    padV2 = tmp1.tile([1, 1], F32, tag="padV2")
                    nc.vector.tensor_copy(padV2, x_t[:1, :1])
                    padP = tmp1.tile([1, 1], F32, tag="padP")
                    nc.gpsimd.tensor_copy(padP, mk_t[:1, :1])

                dl_bf = tmp1.tile([D, CH], BF16, tag="dlbf")
                pd = [None, None]
                sm = [None, None]
                rc = [None, None]

                def mk_mm(mi):
                    def go():
                        pd[mi] = ps_dht.tile([128, CH], F32, tag="pd", name="pd")
                        nc.tensor.matmul(
                            pd[mi],
                            s_wgh[:, mi * 128 : (mi + 1) * 128],
                            dl_bf,
                            start=True,
                            stop=True,
                        )

                    return go

                def mk_sig(mi):
                    def go():
                        sm[mi] = tmp1.tile([128, CH], F32, tag="sg", name="sg")
                        nc.scalar.activation(
                            sm[mi], pd[mi], AF.Sigmoid, bias=s_bgh2[:, mi : mi + 1]
                        )

                    return go

                def mk_rc(mi):
                    def go():
                        rc[mi] = tmp1.tile([128, CH], F32, tag="rc", name="rc")
                        nc.vector.reciprocal_approx_fast(rc[mi], sm[mi])

                    return go

                def mk_ts(mi):
                    def go():
                        nc.vector.tensor_scalar(
                            dht[:, mi, cs : cs + CH], rc[mi], 1.0, 1.0, ALU.subtract, ALU.min
                        )

                    return go

                return [
                    [pads],
                    [lambda: nc.scalar.copy(dl_bf, dl_t)],
                    [mk_mm(0), mk_sig(0)],
                    [mk_rc(0)],
                    [mk_ts(0), mk_mm(1)],
                    [mk_sig(1)],
                    [mk_rc(1)],
                    [mk_ts(1)],
                ]

            def ph1_partB(c):
                """imputation x_hat for chunk c: 8 emission slots.
                dxt = exp(-relu(wgx*Delta+bgx)) = min(1/sigmoid(q)-1, 1)."""
                cs = c * CH
                x_t, xl_t, mk_t, dl_t = ph1_tiles[c]

                # xm broadcast AP: [D, TC1(t), BC(b)] with b-step 0
                xsl = s_xmean[:, c * TC1 : (c + 1) * TC1]
                xm_b = bass.AP(
                    tensor=xsl.tensor,
                    offset=xsl.offset,
                    ap=[xsl.ap[0], xsl.ap[1], [0, BC]],
                )

                def r3(t):
                    return t.rearrange("p (t b) -> p t b", b=BC)

                sx = tmp1.tile([D, CH], F32, tag="sg")
                rcx = tmp1.tile([D, CH], F32, tag="rc")
                dxt = tmp1.tile([D, CH], F32, tag="dxt")
                s1 = tmp1.tile([D, CH], F32, tag="s1")
                s2 = tmp1.tile([D, CH], F32, tag="s2")
                s3 = tmp1.tile([D, CH], F32, tag="s3")
                s4 = tmp1.tile([D, CH], F32, tag="s4")
                s5 = tmp1.tile([D, CH], F32, tag="s5")

                return [
                    [
                        lambda: nc.scalar.activation(
                            sx, dl_t, AF.Sigmoid, bias=s_bgx[:, 0:1], scale=s_wgxd[:, 0:1]
                        )
                    ],
                    [lambda: nc.vector.reciprocal_approx_fast(rcx, sx)],
                    [
                        lambda: nc.vector.tensor_scalar(
                            dxt, rcx, 1.0, 1.0, ALU.subtract, ALU.min
                        )
                    ],
                    [lambda: nc.vector.tensor_sub(r3(s1), r3(xl_t), xm_b)],
                    [lambda: nc.gpsimd.tensor_mul(s2, dxt, s1)],
                    [
                        lambda: nc.vector.tensor_add(r3(s3), r3(s2), xm_b),
                        lambda: nc.scalar.copy(m_bf[:, cs : cs + CH], mk_t),
                    ],
                    [lambda: nc.vector.tensor_sub(s4, x_t, s3)],
                    [
                        lambda: nc.gpsimd.tensor_mul(s5, mk_t, s4),
                        lambda: nc.vector.tensor_add(xhat_bf[:, cs : cs + CH], s3, s5),
                    ],
                ]

            # ===================== group preamble pieces ===================
            pre_tiles = {}

            def preamble_instrs(g):
                """15 matmuls: 3 gate biases + 12 input-dependent terms."""
                pzr = ps_zr.tile([128, 1024], F32)  # banks: z | r
                ph_ = ps_h.tile([128, 512], F32)
                pre_tiles[g] = (pzr, ph_)
                gs = g * GCOLS
                L = []
                L.append(
                    lambda: nc.tensor.matmul(
                        pzr[:, 0:512], s_g["z"]["b2"], s_ones2,
                        start=True, stop=False, skip_group_check=True,
                    )
                )
                L.append(
                    lambda: nc.tensor.matmul(
                        pzr[:, 512:1024], s_g["r"]["b2"], s_ones2,
                        start=True, stop=False, skip_group_check=True,
                    )
                )
                L.append(
                    lambda: nc.tensor.matmul(
                        ph_[:, 0:512], s_g["h"]["b2"], s_ones2,
                        start=True, stop=False, skip_group_check=True,
                    )
                )
                for gname, dst, goff in (("z", pzr, 0), ("r", pzr, 512), ("h", ph_, 0)):
                    for mi in range(2):
                        for wkey, src in (("wx", xhat_bf), ("wm", m_bf)):

                            def go(gname=gname, dst=dst, goff=goff, mi=mi, wkey=wkey, src=src):
                                reg = dst[:, goff + mi * 256 : goff + (mi + 1) * 256]
                                nc.tensor.matmul(
                                    reg,
                                    s_g[gname][wkey][:, mi * 128 : (mi + 1) * 128],
                                    src[:, gs : gs + GCOLS],
                                    start=False, stop=False, skip_group_check=True,
                                )

                            L.append(go)
                return L

            # ========================= scan step ===========================
            state = [gbf0, g0]

            def emit_step(g, tl):
                t = g * TG + tl
                prev_gbf, prev_g = state
                pzr, ph_ = pre_tiles[g]
                pzr4 = pzr.rearrange("p (j q b) -> p j q b", j=4, b=BC)
                ph2 = ph_.rearrange("p (j q b) -> p j q b", j=2, b=BC)

                # r first so sigmoid(r) starts ASAP; z overlaps with it
                for gname, goff in (("r", 2), ("z", 0)):
                    uu = s_g[gname]["u"]
                    for mi in range(2):
                        reg = pzr4[:, goff + mi, tl, :]
                        for k in range(2):
                            nc.tensor.matmul(
                                reg, uu[:, k, mi, :], prev_gbf[:, k, :],
                                start=False, stop=(k == 1), skip_group_check=True,
                            )
                r_t = scan.tile([128, 2, BC], F32, tag="r")
                nc.scalar.activation(r_t, pzr4[:, 2:4, tl, :], AF.Sigmoid)
                z_t = scan.tile([128, 2, BC], F32, tag="z")
                nc.scalar.activation(z_t, pzr4[:, 0:2, tl, :], AF.Sigmoid)

                sbf = scan.tile([128, 2, BC], BF16, tag="sbf")
                nc.vector.tensor_mul(sbf, r_t, prev_g)

                uu = s_g["h"]["u"]
                for mi in range(2):
                    reg = ph2[:, mi, tl, :]
                    for k in range(2):
                        nc.tensor.matmul(
                            reg, uu[:, k, mi, :], sbf[:, k, :],
                            start=False, stop=(k == 1), skip_group_check=True,
                        )
                c_t = scan.tile([128, 2, BC], F32, tag="c")
                nc.scalar.activation(c_t, ph2[:, :, tl, :], AF.Tanh)

                dsl = dht[:, :, (t + 1) * BC : (t + 2) * BC]
                # off-chain terms (Pool + DVE) overlapping PE/ACT work
                w_t = scan.tile([128, 2, BC], F32, tag="w")
                nc.gpsimd.tensor_mul(w_t, dsl, prev_g)
                f_t = scan.tile([128, 2, BC], F32, tag="f")
                nc.vector.tensor_mul(f_t, z_t, dsl)
                en = scan.tile([128, 2, BC], F32, tag="en")
                nc.vector.scalar_tensor_tensor(
                    en, z_t, 1.0, w_t, ALU.subtract, ALU.mult
                )  # (z-1)*w = -(1-z)*dht'*g
                # on-chain tail: u = f*c ; gbf' = u - en
                u_t = scan.tile([128, 2, BC], F32, tag="u")
                nc.vector.tensor_mul(u_t, f_t, c_t)
                gbf_n = scan.tile([128, 2, BC], BF16, tag="gbf")
                nc.vector.tensor_sub(gbf_n, u_t, en)
                g_n = scan.tile([128, 2, BC], F32, tag="g")
                nc.gpsimd.tensor_sub(g_n, u_t, en)
                state[0] = gbf_n
                state[1] = g_n

            # ===================== emission schedule =======================
            ph1_dma(0)
            ph1_dma(1)
            ph1_dma(2)
            for slot in ph1_partA(0):
                for fn in slot:
                    fn()
            for slot in ph1_partB(0):
                for fn in slot:
                    fn()
            for slot in ph1_partA(1):
                for fn in slot:
                    fn()
            for slot in ph1_partB(1):
                for fn in slot:
                    fn()
            for fn in preamble_instrs(0):
                fn()

            for g in range(NG):
                slots = [[] for _ in range(TG)]
                if g + 1 < NG:
                    # 15 preamble matmuls for group g+1: 2/step, 1 at tl=7
                    pl = preamble_instrs(g + 1)
                    for i, fn in enumerate(pl):
                        slots[min(i // 2, 7)].append(fn)
                if g % 2 == 1 and 2 <= (g + 3) // 2 < NCH:
                    for tl, slot in enumerate(ph1_partA((g + 3) // 2)):
                        slots[tl].extend(slot)
                if g % 2 == 0 and 2 <= (g + 2) // 2 < NCH:
                    for tl, slot in enumerate(ph1_partB((g + 2) // 2)):
                        slots[tl].extend(slot)
                if g % 2 == 0 and 3 <= (g + 6) // 2 < NCH:
                    slots[0].append(lambda c=(g + 6) // 2: ph1_dma(c))
                for tl in range(TG):
                    emit_step(g, tl)
                    for fn in slots[tl]:
                        fn()

            # ---- output: out = W_out @ h + b_out  -> [1, BC]
            h_fin = state[1]
            po = ps_out.tile([1, BC], F32)
            for k in range(2):
                nc.tensor.matmul(
                    po, s_wout2[:, k : k + 1], h_fin[:, k, :],
                    start=(k == 0), stop=(k == 1), skip_group_check=True,
                )
            o_sb = scan.tile([1, BC], F32, tag="o")
            nc.scalar.activation(o_sb, po, AF.Identity, bias=s_bout[:, 0:1])
            nc.sync.dma_start(out=out_d[:, :], in_=o_sb)

    nc.compile()  # bacc: splits multi-sem waits into event-semaphore chains
    _BUILT = nc
    return nc


# ------------------------------------------------------------- host wrapper

TRACE = False
LAST_EXEC_NS = None
LAST_RESULT = None


def _host_prep(inputs):
    import ml_dtypes

    bf = ml_dtypes.bfloat16
    inp = np.asarray(inputs["inp"], np.float32)
    X_mean = np.asarray(inputs["X_mean"], np.float32)
    W_z = np.asarray(inputs["W_z"], np.float32)
    b_z = np.asarray(inputs["b_z"], np.float32)
    W_r = np.asarray(inputs["W_r"], np.float32)
    b_r = np.asarray(inputs["b_r"], np.float32)
    W_h = np.asarray(inputs["W_h"], np.float32)
    b_h = np.asarray(inputs["b_h"], np.float32)
    W_gx = np.asarray(inputs["W_gx"], np.float32)
    b_gx = np.asarray(inputs["b_gx"], np.float32)
    W_gh = np.asarray(inputs["W_gh"], np.float32)
    b_gh = np.asarray(inputs["b_gh"], np.float32)
    W_out = np.asarray(inputs["W_out"], np.float32)
    b_out = np.asarray(inputs["b_out"], np.float32)

    def uprep(W):
        U = W[:, D : D + H]  # [256, 256]
        return np.ascontiguousarray(
            U.reshape(2, 128, 2, 128).transpose(3, 2, 0, 1)
        ).astype(bf)

    shared = {
        "xmean_t": np.ascontiguousarray(X_mean[0].T),
        "wgx_diag": np.ascontiguousarray(np.diag(W_gx)).reshape(D, 1),
        "b_gx_c": b_gx.reshape(D, 1),
        "wgh_t": np.ascontiguousarray(W_gh.T).astype(bf),
        "b_gh2": np.ascontiguousarray(b_gh.reshape(2, 128).T),
        "ones2": np.concatenate(
            [
                np.concatenate([np.ones((1, 256)), np.zeros((1, 256))], 1),
                np.concatenate([np.zeros((1, 256)), np.ones((1, 256))], 1),
            ],
            0,
        ).astype(bf),
        "wout2": np.ascontiguousarray(W_out[0].reshape(2, 128).T),
        "b_out_c": b_out.reshape(1, 1),
    }
    for gname, W, bv in (("z", W_z, b_z), ("r", W_r, b_r), ("h", W_h, b_h)):
        shared[f"wx_{gname}"] = np.ascontiguousarray(W[:, :D].T).astype(bf)
        shared[f"wm_{gname}"] = np.ascontiguousarray(W[:, D + H :].T).astype(bf)
        shared[f"u_{gname}"] = uprep(W)
        shared[f"b2_{gname}"] = bv.reshape(2, 128).astype(bf)

    in_maps = []
    for c in range(NCORES):
        sl = inp[c * BC : (c + 1) * BC]  # [BC, 4, T, D]
        arr = np.ascontiguousarray(sl.transpose(1, 3, 2, 0)).reshape(4, D, COLS)
        m = dict(shared)
        m["inp4"] = arr
        in_maps.append(m)
    return in_maps


def kernel(**inputs):
    global LAST_EXEC_NS, LAST_RESULT
    nc = _build()
    in_maps = _host_prep(inputs)
    res = run_bass_kernel_spmd(nc, in_maps, list(range(NCORES)), trace=TRACE)
    LAST_EXEC_NS = res.exec_time_ns
    LAST_RESULT = res
    out = np.concatenate([res.results[c]["out"][0] for c in range(NCORES)])
    return out.reshape(B, 1).astype(np.float32)
